# revision 24
# baseline (speedup 1.0000x reference)
"""Causal self-attention (B=1, T=4096, C=1024, H=16) on 8 TRN2 NeuronCores.

Sharding: tensor-parallel over heads. Core i computes heads (2i, 2i+1):
  - qkv projection for its 384 output columns (computed transposed: [384, T])
  - flash-style causal attention on-chip in S.T ([k,q]) layout; softmax
    denominators fused into the PV matmul via a ones-row in v_aug
  - output projection against its 128 rows of w_proj -> partial y [4096,1024]
Host sums the 8 partial outputs (the tensor-parallel all-reduce).

Schedule (single fused pipeline, all engines busy):
  - quarter loop: x loads + qkv matmuls + v transposes + head-A attention for
    the two q-groups whose data just completed (ACT's exp starts ~20us in)
  - head-B attention loop, with the normalize+projection block for group g-1
    interleaved so the y DMAs overlap head-B's ACT-bound attention
Matmuls use float32r (fp32 data, full-rate PE); fp32 proper is 4 cycles/row.
All matmul operands sit at SBUF base partition 0 (nonzero base would trigger
tiled-PE mode); partition-crossing moves are done with SBUF->SBUF DMA.
Per-(h,g) softmax sums live on PSUM partition 64; head-B reciprocals run
there directly (custom-DVE ops allow 32-aligned base partitions).
"""

import os
import sys

for _p in ("/opt/trn_rl_repo", "/root/.axon_site/_ro/trn_rl_repo"):
    if os.path.isdir(_p) and _p not in sys.path:
        sys.path.insert(0, _p)

import numpy as np

import concourse.bass as bass
import concourse.bacc as bacc
import concourse.mybir as mybir
import concourse.tile as tile
from concourse.bass_utils import run_bass_kernel_spmd

T = 4096
C = 1024
H = 16
HD = 64
NCORES = 8
HPC = H // NCORES  # heads per core = 2
DT = mybir.dt.float32

NG = 8      # q groups of 512
GQ = 512    # q per group
NTC = T // 128  # 32 k/t-chunks
SKEW = 2    # ST-ahead-of-PV software pipeline depth (in work units)


def _build_body(tc, reps=1):
    nc = tc.nc
    RT = mybir.dt.float32r
    xT = nc.dram_tensor("xT", [C, T], RT, kind="ExternalInput").ap()
    wqkvT = nc.dram_tensor("wqkvT", [C, 384], RT, kind="ExternalInput").ap()
    wpT = nc.dram_tensor("wpT", [128, C], RT, kind="ExternalInput").ap()
    maskT = nc.dram_tensor("maskT", [128, 128], RT, kind="ExternalInput").ap()
    ident = nc.dram_tensor("ident", [64, 64], RT, kind="ExternalInput").ap()
    ones = nc.dram_tensor("ones", [1, 64], RT, kind="ExternalInput").ap()
    y = nc.dram_tensor("y", [T, C], DT, kind="ExternalOutput").ap()

    Exp = mybir.ActivationFunctionType.Exp
    ISCALE = 1.0 / float(np.sqrt(HD))

    from contextlib import ExitStack

    with ExitStack() as ctx:
        consts = ctx.enter_context(tc.tile_pool(name="consts", bufs=1))
        qkvsb = ctx.enter_context(tc.tile_pool(name="qkvsb", bufs=1))
        ptpool = ctx.enter_context(tc.tile_pool(name="ptpool", bufs=3))
        stpool = ctx.enter_context(tc.tile_pool(name="stage", bufs=2))
        psA = ctx.enter_context(tc.tile_pool(name="psA", bufs=2, space="PSUM"))
        psST = ctx.enter_context(tc.tile_pool(name="psST", bufs=2, space="PSUM"))
        psOT = ctx.enter_context(tc.tile_pool(name="psOT", bufs=2, space="PSUM"))

        # ---- constants (DMAs deferred below the first x loads) ----
        wp_sb = [consts.tile([64, C], RT, tag=f"wp{h}", name=f"wp{h}")
                 for h in range(HPC)]
        mask_sb = consts.tile([128, 128], RT, tag="mask")
        id_sb = consts.tile([64, 64], RT, tag="ident")
        ones_sb = consts.tile([1, 64], RT, tag="ones")

        def emit_const_dmas():
            for h in range(HPC):
                nc.sync.dma_start(wp_sb[h][:], wpT[64 * h:64 * (h + 1), :])
            nc.sync.dma_start(mask_sb[:], maskT[:, :])
            nc.sync.dma_start(id_sb[:], ident[:, :])
            nc.sync.dma_start(ones_sb[:], ones[:, :])

        # resident activations
        qkvT_sb = qkvsb.tile([128, 2 * T], RT, tag="qkvT")  # headA q|k regions
        qkb_sb = qkvsb.tile([64, 2 * T], RT, tag="qkb")     # head B q|k, base 0
        vaug_sb = qkvsb.tile([128, HPC * NTC * 65], RT, tag="vaug")
        otu_sb = [qkvsb.tile([64, T], RT, tag=f"otu{h}", name=f"otu{h}")
                  for h in range(HPC)]
        sums_sb = qkvsb.tile([NG, GQ], DT, tag="sums0")
        recip_sb = qkvsb.tile([NG, GQ], DT, tag="recip0")
        rscr_sb = qkvsb.tile([NG, GQ], DT, tag="rscr")

        nc.vector.memset(vaug_sb[:].bitcast(mybir.dt.uint32), 0x3F800000)

        def q_ap(h, lo, n):
            if h == 0:
                return qkvT_sb[0:64, lo:lo + n]
            return qkb_sb[:, lo:lo + n]

        def k_ap(h, lo, n):
            if h == 0:
                return qkvT_sb[0:64, T + lo: T + lo + n]
            return qkb_sb[:, T + lo: T + lo + n]

        def attention(h, g, sums_sink):
            """Emit one (head, q-group) of S.T/exp/PV, software-pipelined.
            sums_sink(ot) consumes the softmax sums row; otu copy emitted
            here."""
            ot = psOT.tile([128, GQ], DT, tag="psOT", name="ot")
            units = [("od", kc) for kc in range(0, 4 * g, 2)]
            units += [("dg", 4 * g + j) for j in range(4)]
            nun = len(units)

            def emit_st(u):
                kind, kc = u
                st = psST.tile([128, 2 * GQ], DT, tag="psST", name="st")
                pt = ptpool.tile([128, 2 * GQ], RT, tag="pt", name="pt")
                if kind == "od":
                    for i in range(2):
                        nc.tensor.matmul(st[:, i * GQ:(i + 1) * GQ],
                                         (k_ap(h, (kc + i) * 128, 128)),
                                         (q_ap(h, g * GQ, GQ)),
                                         start=True, stop=True)
                    nc.scalar.activation(pt[:, 0:2 * GQ], st[:, 0:2 * GQ],
                                         Exp, scale=ISCALE)
                else:
                    j = kc - 4 * g
                    nq = GQ - 128 * j
                    qoff = g * GQ + 128 * j
                    nc.tensor.matmul(st[:, 0:nq], (k_ap(h, kc * 128, 128)),
                                     (q_ap(h, qoff, nq)),
                                     start=True, stop=True)
                    nc.scalar.activation(pt[:, 0:nq], st[:, 0:nq], Exp,
                                         scale=ISCALE)
                    nc.vector.tensor_mul(pt[:, 0:128], pt[:, 0:128],
                                         mask_sb[:])
                return pt

            def emit_pv(u, pt, first, last):
                kind, kc = u
                if kind == "od":
                    for i in range(2):
                        voff = (h * NTC + kc + i) * 65
                        nc.tensor.matmul(ot[0:65, :],
                                         (vaug_sb[:, voff:voff + 65]),
                                         (pt[:, i * GQ:(i + 1) * GQ]),
                                         start=(first and i == 0),
                                         stop=(last and i == 1))
                else:
                    j = kc - 4 * g
                    nq = GQ - 128 * j
                    voff = (h * NTC + kc) * 65
                    nc.tensor.matmul(ot[0:65, GQ - nq:GQ],
                                     (vaug_sb[:, voff:voff + 65]),
                                     (pt[:, 0:nq]), start=first, stop=last)

            pts = [None] * nun
            for ui, u in enumerate(units):
                pts[ui] = emit_st(u)
                if ui >= SKEW:
                    k = ui - SKEW
                    emit_pv(units[k], pts[k], k == 0, k == nun - 1)
            for k in range(max(0, nun - SKEW), nun):
                emit_pv(units[k], pts[k], k == 0, k == nun - 1)

            nc.vector.tensor_copy(otu_sb[h][:, g * GQ:(g + 1) * GQ],
                                  ot[0:64, :])
            sums_sink(ot)

        for _rep in range(reps):
            repctx = ExitStack()
            p1pool = repctx.enter_context(tc.tile_pool(name="p1pool", bufs=1))
            xpool = repctx.enter_context(tc.tile_pool(name="xpool", bufs=11))
            vstage = repctx.enter_context(tc.tile_pool(name="vstage", bufs=3))
            vbpool = repctx.enter_context(tc.tile_pool(name="vbpool", bufs=4))

            wq_sb = []
            for cc in range(8):
                t_ = p1pool.tile([128, 384], RT, tag=f"wq{cc}", name=f"wq{cc}")
                nc.sync.dma_start(t_[:], wqkvT[cc * 128:(cc + 1) * 128, :])
                wq_sb.append(t_)
            if _rep == 0:
                emit_const_dmas()

            def h0_sums_sink(g):
                def sink(ot):
                    sst = stpool.tile([65, GQ], DT, tag="sstage", name="sst")
                    nc.vector.tensor_copy(sst[64:65, :], ot[64:65, :])
                    nc.sync.dma_start(sums_sb[g:g + 1, :], sst[64:65, :])
                return sink

            # ---- fused qkv + head-A attention, by quarters of T ----
            for qtr in range(4):
                for tgl in range(2):
                    tg = qtr * 2 + tgl
                    xts = []
                    for cc in range(8):
                        xt = xpool.tile([128, GQ], RT, tag="x")
                        nc.sync.dma_start(
                            xt[:], xT[cc * 128:(cc + 1) * 128,
                                      tg * GQ:(tg + 1) * GQ])
                        xts.append(xt)
                    vst = None
                    for m in range(3):
                        ps = psA.tile([128, GQ], DT, tag="psA", name="ps")
                        for cc in range(8):
                            nc.tensor.matmul(
                                ps[:], (wq_sb[cc][:, m * 128:(m + 1) * 128]),
                                (xts[cc][:]),
                                start=(cc == 0), stop=(cc == 7))
                        if m < 2:
                            lo = m * T + tg * GQ
                            if m == 0:
                                nc.scalar.copy(qkvT_sb[:, lo:lo + GQ], ps[:])
                            else:
                                nc.vector.tensor_copy(qkvT_sb[:, lo:lo + GQ],
                                                      ps[:])
                            nc.sync.dma_start(qkb_sb[:, lo:lo + GQ],
                                              qkvT_sb[64:128, lo:lo + GQ])
                        else:
                            vst = vstage.tile([128, GQ], RT, tag="vst",
                                              name="vst")
                            nc.vector.tensor_copy(vst[:], ps[:])
                    # v natural layout for the 4 k-chunks of this tg
                    vb_t = vbpool.tile([64, GQ], RT, tag="vb", name="vb")
                    nc.sync.dma_start(vb_t[:], vst[64:128, :])
                    for kcl in range(4):
                        kc = tg * 4 + kcl
                        for h in range(HPC):
                            vt = psA.tile([128, 64], RT, tag="psA", name="vt")
                            if h == 0:
                                src = vst[0:64, kcl * 128:(kcl + 1) * 128]
                            else:
                                src = vb_t[:, kcl * 128:(kcl + 1) * 128]
                            nc.tensor.transpose(vt[:], src, id_sb[:])
                            off = (h * NTC + kc) * 65
                            nc.vector.tensor_copy(vaug_sb[:, off:off + 64],
                                                  vt[:])
                # head-A attention for the two groups this quarter completed
                for g in (2 * qtr, 2 * qtr + 1):
                    attention(0, g, h0_sums_sink(g))

            repctx.close()
            postctx = ExitStack()
            rinpool = postctx.enter_context(tc.tile_pool(name="rinpool",
                                                         bufs=6))
            rbpool = postctx.enter_context(tc.tile_pool(name="rbpool", bufs=2))
            ypool = postctx.enter_context(tc.tile_pool(name="ypool", bufs=3))

            nc.vector.reciprocal_approx_accurate(recip_sb[:], sums_sb[:],
                                                 rscr_sb[:])

            rins = [None] * NG  # head-B per-group reciprocal rows (base 0)

            def h1_sums_sink(g):
                def sink(ot):
                    sst = stpool.tile([65, GQ], DT, tag="sstage", name="sst")
                    nc.vector.tensor_copy(sst[64:65, :], ot[64:65, :])
                    s0 = rinpool.tile([1, GQ], DT, tag="rin", name="s0")
                    nc.sync.dma_start(s0[:], sst[64:65, :])
                    rin = rinpool.tile([1, GQ], DT, tag="rin", name="rin")
                    sc = rinpool.tile([1, GQ], DT, tag="rin", name="sc")
                    nc.vector.reciprocal_approx_accurate(rin[:], s0[:], sc[:])
                    rins[g] = rin
                return sink

            def proj_block(g):
                # normalize both heads' O.T for group g (in place), then
                # project: per-head K=64 matmuls accumulate in PSUM
                rin0 = rinpool.tile([1, GQ], DT, tag="rin", name="rin0")
                nc.sync.dma_start(rin0[:], recip_sb[g:g + 1, :])
                for h, rv in ((0, rin0), (1, rins[g])):
                    rvr = rinpool.tile([1, GQ], RT, tag="rin", name="rvr")
                    nc.vector.tensor_copy(rvr[:], rv[:])
                    rb = psA.tile([128, GQ], DT, tag="psA", name="rb")
                    nc.tensor.matmul(rb[0:64, :], ones_sb[:], rvr[:],
                                     start=True, stop=True)
                    rbs = rbpool.tile([64, GQ], RT, tag="rb")
                    nc.vector.tensor_copy(rbs[:], rb[0:64, :])
                    nc.vector.tensor_mul(otu_sb[h][:, g * GQ:(g + 1) * GQ],
                                         otu_sb[h][:, g * GQ:(g + 1) * GQ],
                                         rbs[:])
                for t2 in range(4):
                    t0 = g * GQ + t2 * 128
                    for og in range(2):
                        yp = psA.tile([128, GQ], DT, tag="psA", name="yp")
                        for h in range(HPC):
                            nc.tensor.matmul(
                                yp[:], (otu_sb[h][:, t0:t0 + 128]),
                                (wp_sb[h][:, og * GQ:(og + 1) * GQ]),
                                start=(h == 0), stop=(h == 1))
                        ysb = ypool.tile([128, GQ], DT, tag="ysb")
                        nc.vector.tensor_copy(ysb[:], yp[:])
                        nc.sync.dma_start(
                            y[t0:t0 + 128, og * GQ:(og + 1) * GQ], ysb[:])

            # ---- head-B attention with interleaved projection ----
            for g in range(NG):
                attention(1, g, h1_sums_sink(g))
                if g >= 1:
                    proj_block(g - 1)
            proj_block(NG - 1)
            postctx.close()


_CACHE = {}


def build_module(reps=1):
    key = ("nc", reps)
    if key not in _CACHE:
        nc = bacc.Bacc("TRN2", target_bir_lowering=False, debug=False)
        with tile.TileContext(nc) as tc:
            _build_body(tc, reps=reps)
        nc.compile()
        _CACHE[key] = nc
    return _CACHE[key]


def _host_prep(x, w_attn, w_proj):
    x = np.asarray(x, dtype=np.float32)
    w_attn = np.asarray(w_attn, dtype=np.float32)
    w_proj = np.asarray(w_proj, dtype=np.float32)
    X = x.reshape(T, C)
    xTh = np.ascontiguousarray(X.T)
    mask = np.triu(np.ones((128, 128), dtype=np.float32))  # mask[k,q]=1 iff q>=k
    eye = np.eye(64, dtype=np.float32)
    ones = np.ones((1, 64), dtype=np.float32)
    Wq, Wk, Wv = w_attn[0:C], w_attn[C:2 * C], w_attn[2 * C:3 * C]
    in_maps = []
    for i in range(NCORES):
        hA, hB = 2 * i, 2 * i + 1
        Wc = np.concatenate([
            Wq[64 * hA:64 * hA + 64], Wq[64 * hB:64 * hB + 64],
            Wk[64 * hA:64 * hA + 64], Wk[64 * hB:64 * hB + 64],
            Wv[64 * hA:64 * hA + 64], Wv[64 * hB:64 * hB + 64],
        ], axis=0)  # [384, C]
        in_maps.append({
            "xT": xTh,
            "wqkvT": np.ascontiguousarray(Wc.T),
            "wpT": np.ascontiguousarray(w_proj[:, 128 * i:128 * (i + 1)].T),
            "maskT": mask,
            "ident": eye,
            "ones": ones,
        })
    return in_maps


def run(x, w_attn, w_proj, trace=False):
    nc = build_module()
    in_maps = _host_prep(x, w_attn, w_proj)
    res = run_bass_kernel_spmd(nc, in_maps, core_ids=list(range(NCORES)),
                               trace=trace)
    parts = np.stack([r["y"] for r in res.results], axis=0)
    yfull = parts.sum(axis=0, dtype=np.float64).astype(np.float32)
    return yfull.reshape(1, T, C), res


def kernel(x, w_attn, w_proj):
    yfull, _ = run(x, w_attn, w_proj, trace=False)
    return yfull
